# revision 2
# baseline (speedup 1.0000x reference)
"""Circular relative-position attention on 8 trn2 NeuronCores.

Algorithm (per (batch,head), S=1024, hd=64):
  scores[q,k] = dot(Q[q],K[k])/8 + dot(Q[q], Wk[(k-q)%S]),  Wk[u] = rel_pos_k[min(u,S-u)]
  attn = softmax_k(scores)
  out[q] = sum_k attn[q,k]*V[k] + sum_u attn[q,(q+u)%S]*Wv[u],  Wv[u] = rel_pos_v[min(u,S-u)]

Kernel layout choices:
  - scores live (q_part x k_free); Q@Wk^T is computed in "skewed" coords
    (q x u) where it is a plain matmul, then sheared to (q x k) via a
    per-q-tile diagonal DRAM buffer (write once diagonally, read back
    diagonally -- circularity linearized by a 256-col pad + small dup band).
  - rel term is accumulated into the scores PSUM with an identity matmul.
  - exp via ACT with scale=1/8 folded in (host pre-scales Wk by 8), row
    sums via ACT accum_out; normalize with tensor_scalar mult.
  - normalized attn (bf16) is written twice side-by-side to DRAM (1024x2048);
    xbar-transpose DMAs read back P^T (straight columns) for attn@V and
    attn_skew^T (diagonal columns) for the rel_v matmul; both accumulate
    into one (64x1024) PSUM out^T which is stored per (b,h).
  - batch is sharded across the 8 cores (1 batch each, 16 heads).
"""

import os
import sys
import numpy as np

for _p in ("/opt/trn_rl_repo", "/root/.axon_site/_ro/trn_rl_repo"):
    if os.path.isdir(_p) and _p not in sys.path:
        sys.path.insert(0, _p)

import ml_dtypes
from contextlib import ExitStack

import concourse.bass as bass
import concourse.tile as tile
from concourse import bacc, mybir
from concourse.masks import make_identity

FP32 = mybir.dt.float32
F32R = mybir.dt.float32r
BF16 = mybir.dt.bfloat16

B, S, D, H = 8, 1024, 1024, 16
HD = D // H
NCORES = 8


def build_module(nbh=H, s=S, hd=HD):
    """Build the per-core Bass module. nbh (b,h) pairs, seq len s."""
    nt = s // 128            # q/k tiles
    w1 = s + 256             # per-tile shear buffer width
    ch = min(512, s)         # matmul free-dim chunk
    nc = bacc.Bacc("TRN2", target_bir_lowering=False, debug=False)

    qT = nc.dram_tensor("qT", [nbh, hd, s], F32R, kind="ExternalInput")
    kT = nc.dram_tensor("kT", [nbh, hd, s], F32R, kind="ExternalInput")
    v = nc.dram_tensor("v", [nbh, s, hd], BF16, kind="ExternalInput")
    wT = nc.dram_tensor("wT", [hd, s], F32R, kind="ExternalInput")
    wv = nc.dram_tensor("wv", [s, hd], BF16, kind="ExternalInput")
    outT = nc.dram_tensor("outT", [nbh, hd, s], FP32, kind="ExternalOutput")

    relbuf = [
        [nc.dram_tensor(f"relbuf_{pp}_{t}", [128, w1], BF16, kind="Internal")
         for t in range(nt)]
        for pp in range(nbh)
    ]
    attq = [nc.dram_tensor(f"attq_{pp}", [s, 2 * s], BF16, kind="Internal")
            for pp in range(nbh)]

    def dap(tensor, offset, pattern):
        return bass.AP(tensor, offset, pattern)

    with tile.TileContext(nc) as tc, ExitStack() as ctx:
        const_pool = ctx.enter_context(tc.tile_pool(name="const", bufs=1))
        qkv_pool = ctx.enter_context(tc.tile_pool(name="qkv", bufs=2))
        big_pool = ctx.enter_context(tc.tile_pool(name="big", bufs=3))
        sm_pool = ctx.enter_context(tc.tile_pool(name="small", bufs=4))
        tp_pool = ctx.enter_context(tc.tile_pool(name="tp", bufs=nt + 2))
        ps_big = ctx.enter_context(tc.tile_pool(name="psb", bufs=2, space="PSUM"))
        ps_out = ctx.enter_context(tc.tile_pool(name="pso", bufs=2, space="PSUM"))

        ident = const_pool.tile([128, 128], BF16)
        make_identity(nc, ident[:])
        wT_sb = const_pool.tile([hd, s], F32R)
        nc.sync.dma_start(wT_sb[:], wT.ap())
        wv_sb = const_pool.tile([128, nt * hd], BF16)
        # wv (s, hd) -> tiles (128, hd) side by side: partition j, free (t, d)
        nc.sync.dma_start(
            wv_sb[:], dap(wv, 0, [[hd, 128], [128 * hd, nt], [1, hd]]))

        tp_sem = nc.alloc_semaphore("tp_sem")
        for bh in range(nbh):
            pp = bh
            qT_sb = qkv_pool.tile([hd, s], F32R, tag="qT")
            nc.sync.dma_start(qT_sb[:], qT.ap()[bh])
            kT_sb = qkv_pool.tile([hd, s], F32R, tag="kT")
            nc.sync.dma_start(kT_sb[:], kT.ap()[bh])
            v_sb = qkv_pool.tile([128, nt * hd], BF16, tag="v")
            nc.sync.dma_start(
                v_sb[:],
                dap(v, bh * s * hd, [[hd, 128], [128 * hd, nt], [1, hd]]))

            # ---- Phase A: rel_skew = Q @ (8*Wk)^T, sheared out to DRAM ----
            for t in range(nt):
                psA = ps_big.tile([128, s], FP32, tag="big")
                lhs = qT_sb[:, t * 128:(t + 1) * 128]
                for h0 in range(0, s, ch):
                    nc.tensor.matmul(
                        psA[:, h0:h0 + ch], lhs,
                        wT_sb[:, h0:h0 + ch],
                        start=True, stop=True)
                relb = big_pool.tile([128, s + 128], BF16, tag="relb")
                nc.vector.tensor_copy(relb[:, 0:s], psA[:])
                nc.vector.tensor_copy(relb[:, s:s + 128], psA[:, 0:128])
                # one diagonal write: buf[i, i+u'] = relb[i, u'], u' in [0, s+128)
                # (cols [s, s+128) duplicate cols [0, 128) -> linearizes the wrap)
                nc.sync.dma_start(
                    dap(relbuf[pp][t], 0, [[w1 + 1, 128], [1, s + 128]]),
                    relb[:])

            # ---- Phase B: scores + rel add + exp + normalize + attq ----
            for t in range(nt):
                q0 = t * 128
                psB = ps_big.tile([128, s], FP32, tag="big")
                lhs = qT_sb[:, q0:q0 + 128]
                for h0 in range(0, s, ch):
                    nc.tensor.matmul(
                        psB[:, h0:h0 + ch], lhs,
                        kT_sb[:, h0:h0 + ch],
                        start=True, stop=False)
                # shear read back: rb[i, k] = rel_skew[q0+i, (k - q0 - i) % s]
                rb = big_pool.tile([128, s], BF16, tag="rb")
                nc.sync.dma_start(
                    rb[:, 0:q0 + 128],
                    dap(relbuf[pp][t], s - q0, [[w1, 128], [1, q0 + 128]]))
                if q0 + 128 < s:
                    nc.sync.dma_start(
                        rb[:, q0 + 128:s],
                        dap(relbuf[pp][t], 128, [[w1, 128], [1, s - q0 - 128]]))
                for h0 in range(0, s, ch):
                    nc.tensor.matmul(
                        psB[:, h0:h0 + ch], ident[:], rb[:, h0:h0 + ch],
                        start=False, stop=True)
                pexp = big_pool.tile([128, s], BF16, tag="pexp")
                dnt = sm_pool.tile([128, 1], FP32, tag="dnt")
                nc.scalar.activation(
                    pexp[:], psB[:], mybir.ActivationFunctionType.Exp,
                    scale=0.125, accum_out=dnt[:])
                rec = sm_pool.tile([128, 1], FP32, tag="rec")
                nc.vector.reciprocal(rec[:], dnt[:])
                pn = big_pool.tile([128, s], BF16, tag="pn")
                nc.vector.tensor_scalar_mul(pn[:], pexp[:], rec[:])
                nc.sync.dma_start(
                    dap(attq[pp], q0 * 2 * s, [[2 * s, 128], [1, 2 * s]]),
                    bass.AP(pn[:].tensor, pn[:].offset, [[s, 128], [0, 2], [1, s]]))

            # ---- Phase C: transpose reads + output matmuls ----
            # The xbar transpose DMAs ride a different HW ring than the
            # plain-DMA attq writes; Tile's cross-lane completion model is
            # unsound there, so gate them behind a critical section whose
            # entry waits on the global clock (real semaphores).
            psO = ps_out.tile([hd, s], FP32, tag="out")
            ptts, dsts = [], []
            with tc.tile_critical():
                for j in range(nt):
                    ptt = tp_pool.tile([128, s], BF16, tag="ptt")
                    nc.scalar.dma_start_transpose(
                        ptt[:], dap(attq[pp], j * 128, [[2 * s, s], [1, 128]])
                    ).then_inc(tp_sem, 16)
                    dst = tp_pool.tile([128, s], BF16, tag="dst")
                    nc.scalar.dma_start_transpose(
                        dst[:], dap(attq[pp], j * 128,
                                    [[2 * s + 1, s], [1, 128]])
                    ).then_inc(tp_sem, 16)
                    ptts.append(ptt)
                    dsts.append(dst)
                nc.scalar.wait_ge(tp_sem, 16 * 2 * nt * (bh + 1))
            for j in range(nt):
                ptt, dst = ptts[j], dsts[j]
                vj = v_sb[:, j * hd:(j + 1) * hd]
                wvj = wv_sb[:, j * hd:(j + 1) * hd]
                for h0 in range(0, s, ch):
                    nc.tensor.matmul(
                        psO[:, h0:h0 + ch], vj, ptt[:, h0:h0 + ch],
                        start=(j == 0), stop=False)
                    nc.tensor.matmul(
                        psO[:, h0:h0 + ch], wvj, dst[:, h0:h0 + ch],
                        start=False, stop=(j == nt - 1))
            outsb = big_pool.tile([hd, s], FP32, tag="outsb")
            nc.vector.tensor_copy(outsb[:], psO[:])
            nc.sync.dma_start(outT.ap()[bh], outsb[:])

    nc.compile()
    return nc


_NC_CACHE = {}


def _get_module(nbh, s, hd):
    key = (nbh, s, hd)
    if key not in _NC_CACHE:
        _NC_CACHE[key] = build_module(nbh, s, hd)
    return _NC_CACHE[key]


def _prep_core_inputs(query, key, value, rel_pos_k, rel_pos_v):
    """Host-side shard + layout prep. Returns (in_maps, shared check data)."""
    u = np.arange(S)
    g = np.minimum(u, S - u)
    wT = (rel_pos_k[g] * 8.0).T.astype(np.float32).copy()          # (hd, S)
    wv = rel_pos_v[g].astype(ml_dtypes.bfloat16).copy()            # (S, hd)

    in_maps = []
    for c in range(NCORES):
        q_c = query[c].reshape(S, H, HD)
        k_c = key[c].reshape(S, H, HD)
        v_c = value[c].reshape(S, H, HD)
        in_maps.append({
            "qT": np.ascontiguousarray(q_c.transpose(1, 2, 0)).astype(np.float32),
            "kT": np.ascontiguousarray(k_c.transpose(1, 2, 0)).astype(np.float32),
            "v": np.ascontiguousarray(v_c.transpose(1, 0, 2)).astype(ml_dtypes.bfloat16),
            "wT": wT,
            "wv": wv,
        })
    return in_maps


def _postprocess_core(result_map):
    outT = result_map["outT"]                     # (H, HD, S)
    return outT.transpose(2, 0, 1).reshape(S, D)


def kernel(**inputs):
    from concourse.bass_utils import run_bass_kernel_spmd

    query = np.asarray(inputs["query"], dtype=np.float32)
    key = np.asarray(inputs["key"], dtype=np.float32)
    value = np.asarray(inputs["value"], dtype=np.float32)
    rel_pos_k = np.asarray(inputs["rel_pos_k"], dtype=np.float32)
    rel_pos_v = np.asarray(inputs["rel_pos_v"], dtype=np.float32)

    nc = _get_module(H, S, HD)
    in_maps = _prep_core_inputs(query, key, value, rel_pos_k, rel_pos_v)
    res = run_bass_kernel_spmd(nc, in_maps, core_ids=list(range(NCORES)))

    out = np.empty((B, S, D), dtype=np.float32)
    for c in range(NCORES):
        out[c] = _postprocess_core(res.results[c])
    return out


if __name__ == "__main__":
    rng = np.random.default_rng(0)
    ins = {
        "query": rng.standard_normal((B, S, D)).astype(np.float32),
        "key": rng.standard_normal((B, S, D)).astype(np.float32),
        "value": rng.standard_normal((B, S, D)).astype(np.float32),
        "rel_pos_k": (rng.standard_normal((S, HD)) * 0.02).astype(np.float32),
        "rel_pos_v": (rng.standard_normal((S, HD)) * 0.02).astype(np.float32),
    }
    out = kernel(**ins)
    print("out", out.shape, out.dtype, np.abs(out).max())

